# revision 6
# baseline (speedup 1.0000x reference)
"""Trainium2 Bass kernel for nn_AttentionHeader (GAT-style attention head).

Math:
  seq_fts = seq @ W0                      [N, D]
  f1 = seq_fts @ w1 + b1 ; f2 = seq_fts @ w2 + b2
  logits[i,j] = f1[i] + f2[j]             (rank-1 structure!)
  coefs = softmax(leaky_relu(logits, .2), axis=-1)
  out = coefs @ seq_fts + bias

Key identities used on device (g1 = f1 + b1 + b2, x = g1_i + f2_j):
  exp(lrelu(x)) = max(exp(x), exp(0.2 x))
                = exp(0.2 g1_i) * exp(f2_j) * max(exp(0.8 g1_i), exp(-0.8 f2_j))
Softmax normalizes per row i, so the exp(0.2 g1_i) factor cancels. With
  m_i = exp(0.8 g1_i),  a_j = exp(f2_j),  c_j = exp(-0.8 f2_j):
  coefs_ij  ∝  a_j * max(m_i, c_j)
  out_i = (sum_j max(m_i,c_j) * (a_j s_j)) / (sum_j max(m_i,c_j) a_j) + bias

So the NxN attention matrix never touches HBM, and each [128, 1024] tile
of it (j on partitions, i free) is ONE DVE tensor_scalar_max (2x perf
mode) against the replicated m tile, immediately contracted on the PE
against the a-scaled seq_fts (extra a_j column yields denominators).

Sharding: rows (query nodes) split across 8 cores; seq^T replicated so
each core computes the full seq_fts j-side on the fly.
"""

import sys

if "/opt/trn_rl_repo" not in sys.path:
    sys.path.insert(0, "/opt/trn_rl_repo")

import numpy as np

N = 8192
F = 256
D = 64
NCORES = 8
R = N // NCORES      # 1024 rows per core
P = 128
NJ = N // P          # 64 j-chunks
RI = R // P          # 8 i-subtiles per core

_prog_cache = {}


def _build_program():
    if "nc" in _prog_cache:
        return _prog_cache["nc"]

    import concourse.bacc as bacc
    import concourse.mybir as mybir
    import concourse.tile as tile
    from concourse.masks import make_identity
    from contextlib import ExitStack

    fp32 = mybir.dt.float32
    AF = mybir.ActivationFunctionType
    OP = mybir.AluOpType

    nc = bacc.Bacc(
        "TRN2",
        target_bir_lowering=False,
        debug=False,
        enable_asserts=False,
        num_devices=NCORES,
    )

    seqT = nc.dram_tensor("seqT", [F, N], fp32, kind="ExternalInput").ap()
    ra = nc.dram_tensor("ra", [F, D + 2], fp32, kind="ExternalInput").ap()
    ownT = nc.dram_tensor("ownT", [F, R], fp32, kind="ExternalInput").ap()
    b12 = nc.dram_tensor("b12", [1, 1], fp32, kind="ExternalInput").ap()
    biasv = nc.dram_tensor("biasv", [1, D], fp32, kind="ExternalInput").ap()
    out = nc.dram_tensor("out", [R, D], fp32, kind="ExternalOutput").ap()

    with tile.TileContext(nc) as tc:
        with ExitStack() as ctx:
            const = ctx.enter_context(tc.tile_pool(name="const", bufs=1))
            persist = ctx.enter_context(tc.tile_pool(name="persist", bufs=1))
            stp = ctx.enter_context(tc.tile_pool(name="stp", bufs=6))
            sqp = ctx.enter_context(tc.tile_pool(name="sqp", bufs=4))
            vp = ctx.enter_context(tc.tile_pool(name="vp", bufs=4))
            colp = ctx.enter_context(tc.tile_pool(name="colp", bufs=6))
            obp = ctx.enter_context(tc.tile_pool(name="obp", bufs=3))
            psp = ctx.enter_context(tc.tile_pool(name="psp", bufs=3, space="PSUM"))
            pvp = ctx.enter_context(tc.tile_pool(name="pvp", bufs=1, space="PSUM"))
            scrp = ctx.enter_context(tc.tile_pool(name="scrp", bufs=3, space="PSUM"))

            # ---- constants / parameters ----
            ra0 = const.tile([P, D + 2], fp32, name="ra0")
            ra1 = const.tile([P, D + 2], fp32, name="ra1")
            nc.sync.dma_start(ra0[:, :], ra[0:P, :])
            nc.sync.dma_start(ra1[:, :], ra[P : 2 * P, :])
            b12_sb = const.tile([1, 1], fp32, name="b12_sb")
            nc.sync.dma_start(b12_sb[:, :], b12[:, :])
            bias_sb = const.tile([1, D], fp32, name="bias_sb")
            nc.sync.dma_start(bias_sb[:, :], biasv[:, :])
            ones_row = const.tile([1, P], fp32, name="ones_row")
            nc.vector.memset(ones_row[:, :], 1.0)
            ident = const.tile([P, P], fp32, name="ident")
            make_identity(nc, ident[:, :])

            ot0 = const.tile([P, R], fp32, name="ot0")
            ot1 = const.tile([P, R], fp32, name="ot1")
            nc.sync.dma_start(ot0[:, :], ownT[0:P, :])
            nc.sync.dma_start(ot1[:, :], ownT[P : 2 * P, :])

            # ---- prologue: g1 row for own block, replicated m tile ----
            g1row = persist.tile([1, R], fp32, name="g1row")
            m_rep = persist.tile([P, R], fp32, name="m_rep")
            bias_rep = persist.tile([P, D], fp32, name="bias_rep")

            for h in range(2):
                pf = scrp.tile([P, 512], fp32, name=f"pf{h}", tag="scr")
                cs = slice(h * 512, (h + 1) * 512)
                nc.tensor.matmul(
                    pf[0:1, :], ra0[:, D : D + 1], ot0[:, cs], start=True, stop=False
                )
                nc.tensor.matmul(
                    pf[0:1, :], ra1[:, D : D + 1], ot1[:, cs], start=False, stop=True
                )
                # g1 = f1 + (b1 + b2)
                nc.scalar.activation(
                    g1row[0:1, cs], pf[0:1, :], AF.Identity, bias=b12_sb[0:1, 0:1]
                )
            # broadcast to 128 partitions, m = exp(0.8 g1)
            for h in range(2):
                pb = scrp.tile([P, 512], fp32, name=f"pb{h}", tag="scr")
                cs = slice(h * 512, (h + 1) * 512)
                nc.tensor.matmul(
                    pb[:, :], ones_row[:, :], g1row[0:1, cs], start=True, stop=True
                )
                nc.scalar.activation(m_rep[:, cs], pb[:, :], AF.Exp, scale=0.8)

            pbias = scrp.tile([P, 512], fp32, name="pbias", tag="scr")
            nc.tensor.matmul(
                pbias[:, 0:D], ones_row[:, :], bias_sb[0:1, :], start=True, stop=True
            )
            nc.vector.tensor_copy(bias_rep[:, :], pbias[:, 0:D])

            # ---- accumulators for vals^T ([a*seq_fts | a] contracted with w) ----
            pv0 = pvp.tile([D + 1, 512], fp32, name="pv0", tag="pv0")
            pv1 = pvp.tile([D + 1, 512], fp32, name="pv1", tag="pv1")

            # ---- main loop over j-chunks ----
            for jc in range(NJ):
                js = slice(jc * P, (jc + 1) * P)

                st0 = stp.tile([P, P], fp32, name=f"st0_{jc}", tag="st0")
                st1 = stp.tile([P, P], fp32, name=f"st1_{jc}", tag="st1")
                nc.sync.dma_start(st0[:, :], seqT[0:P, js])
                nc.sync.dma_start(st1[:, :], seqT[P : 2 * P, js])

                # seq_fts chunk + f1 + f2 columns: [128 j, 66]
                ps = psp.tile([P, D + 2], fp32, name=f"ps_{jc}", tag="ps")
                nc.tensor.matmul(ps[:, :], st0[:, :], ra0[:, :], start=True, stop=False)
                nc.tensor.matmul(ps[:, :], st1[:, :], ra1[:, :], start=False, stop=True)

                f2c = ps[:, D + 1 : D + 2]
                a_col = colp.tile([P, 1], fp32, name=f"a_{jc}", tag="a")
                c_col = colp.tile([P, 1], fp32, name=f"c_{jc}", tag="c")
                nc.scalar.activation(a_col[:, :], f2c, AF.Exp)
                nc.scalar.activation(c_col[:, :], f2c, AF.Exp, scale=-0.8)

                # sq = [a * seq_fts | a]
                sq = sqp.tile([P, D + 1], fp32, name=f"sq_{jc}", tag="sq")
                nc.scalar.activation(sq[:, 0:D], ps[:, 0:D], AF.Copy, scale=a_col[:, :])
                nc.vector.tensor_copy(sq[:, D : D + 1], a_col[:, :])

                # w = max(m_i, c_j): one DVE tensor_scalar (2x perf mode)
                w = vp.tile([P, R], fp32, name=f"w_{jc}", tag="w")
                nc.vector.tensor_scalar_max(w[:, :], m_rep[:, :], c_col[:, :])

                first = jc == 0
                last = jc == NJ - 1
                nc.tensor.matmul(
                    pv0[:, :], sq[:, :], w[:, 0:512], start=first, stop=last
                )
                nc.tensor.matmul(
                    pv1[:, :], sq[:, :], w[:, 512:1024], start=first, stop=last
                )

            # ---- epilogue: transpose [65, 1024] -> 8x [128, 65], normalize ----
            vt = persist.tile([D + 1, R], fp32, name="vt")
            nc.scalar.activation(vt[:, 0:512], pv0[:, :], AF.Copy)
            nc.scalar.activation(vt[:, 512:1024], pv1[:, :], AF.Copy)

            for it in range(RI):
                cs = slice(it * P, (it + 1) * P)
                tp = psp.tile([P, D + 2], fp32, name=f"tp_{it}", tag="ps")
                nc.tensor.transpose(
                    tp[:, 0 : D + 1], vt[:, cs], ident[0 : D + 1, 0 : D + 1]
                )
                recip = colp.tile([P, 1], fp32, name=f"r_{it}", tag="r")
                nc.vector.reciprocal(recip[:, :], tp[:, D : D + 1])
                ob = obp.tile([P, D], fp32, name=f"ob_{it}", tag="ob")
                # out = vals_T * (1/denom) + bias
                nc.vector.scalar_tensor_tensor(
                    ob[:, :],
                    tp[:, 0:D],
                    recip[:, :],
                    bias_rep[:, :],
                    op0=OP.mult,
                    op1=OP.add,
                )
                nc.sync.dma_start(out[cs, :], ob[:, :])

    nc.compile()
    _prog_cache["nc"] = nc
    return nc


def _prep_inputs(seq, W0, w1, b1, w2, b2, bias):
    seq = np.asarray(seq, dtype=np.float32)
    W0 = np.asarray(W0, dtype=np.float32)
    w1 = np.asarray(w1, dtype=np.float32).reshape(D, 1)
    w2 = np.asarray(w2, dtype=np.float32).reshape(D, 1)
    b1 = np.asarray(b1, dtype=np.float32).reshape(-1)
    b2 = np.asarray(b2, dtype=np.float32).reshape(-1)
    bias = np.asarray(bias, dtype=np.float32).reshape(1, D)

    seqT = np.ascontiguousarray(seq.reshape(N, F).T)          # [F, N]
    ra = np.ascontiguousarray(
        np.concatenate([W0, W0 @ w1, W0 @ w2], axis=1)        # [F, D+2]
    )
    b12 = np.array([[b1[0] + b2[0]]], dtype=np.float32)

    in_maps = []
    for c in range(NCORES):
        ownT = np.ascontiguousarray(seqT[:, c * R : (c + 1) * R])
        in_maps.append(
            {"seqT": seqT, "ra": ra, "ownT": ownT, "b12": b12, "biasv": bias}
        )
    return in_maps


def run(inputs, trace=False):
    """Returns (output [1, N, D] float32, BassKernelResults)."""
    from concourse import bass_utils

    nc = _build_program()
    in_maps = _prep_inputs(**inputs)
    res = bass_utils.run_bass_kernel_spmd(
        nc, in_maps, core_ids=list(range(NCORES)), trace=trace
    )
    blocks = [res.results[c]["out"] for c in range(NCORES)]
    full = np.concatenate(blocks, axis=0).astype(np.float32)[None]  # [1, N, D]
    return full, res


def kernel(seq, W0, w1, b1, w2, b2, bias):
    out, _ = run(
        {
            "seq": seq,
            "W0": W0,
            "w1": w1,
            "b1": b1,
            "w2": w2,
            "b2": b2,
            "bias": bias,
        }
    )
    return out


# revision 12
# speedup vs baseline: 1.5120x; 1.5120x over previous
"""Trainium2 Bass kernel for nn_AttentionHeader (GAT-style attention head).

Math:
  seq_fts = seq @ W0                      [N, D]
  f1 = seq_fts @ w1 + b1 ; f2 = seq_fts @ w2 + b2
  logits[i,j] = f1[i] + f2[j]             (rank-1 structure!)
  coefs = softmax(leaky_relu(logits, .2), axis=-1)
  out = coefs @ seq_fts + bias

Key identities used on device (g1 = f1 + b1 + b2, x = g1_i + f2_j):
  exp(lrelu(x)) = max(exp(x), exp(0.2 x))
                = exp(0.2 g1_i) * exp(f2_j) * max(exp(0.8 g1_i), exp(-0.8 f2_j))
Softmax normalizes per row i, so the exp(0.2 g1_i) factor cancels. With
  m_i = exp(0.8 g1_i),  a_j = exp(f2_j),  c_j = exp(-0.8 f2_j):
  coefs_ij  ∝  a_j * max(m_i, c_j)
  out_i = (sum_j max(m_i,c_j) * (a_j s_j)) / (sum_j max(m_i,c_j) a_j) + bias

So the NxN attention matrix never touches HBM, and each [128, 1024] tile
of it (j on partitions, i free) is ONE DVE tensor_scalar_max (2x perf
mode) against the replicated m tile, immediately contracted on the PE
against the a-scaled seq_fts (extra a_j column yields denominators).

Sharding: rows (query nodes) split across 8 cores; seq^T replicated so
each core computes the full seq_fts j-side on the fly.
"""

import sys

if "/opt/trn_rl_repo" not in sys.path:
    sys.path.insert(0, "/opt/trn_rl_repo")

import numpy as np

N = 8192
F = 256
D = 64
NCORES = 8
R = N // NCORES      # 1024 rows per core
P = 128
NJ = N // P          # 64 j-chunks
RI = R // P          # 8 i-subtiles per core

_prog_cache = {}


def _build_program():
    if "nc" in _prog_cache:
        return _prog_cache["nc"]

    import concourse.bacc as bacc
    import concourse.mybir as mybir
    import concourse.tile as tile
    from concourse.masks import make_identity
    from contextlib import ExitStack

    fp32 = mybir.dt.float32
    fp16 = mybir.dt.float16
    AF = mybir.ActivationFunctionType
    OP = mybir.AluOpType

    nc = bacc.Bacc(
        "TRN2",
        target_bir_lowering=False,
        debug=False,
        enable_asserts=False,
        num_devices=NCORES,
    )

    seqT = nc.dram_tensor("seqT", [F, N], fp32, kind="ExternalInput").ap()
    ra = nc.dram_tensor("ra", [F, D + 2], fp32, kind="ExternalInput").ap()
    ownT = nc.dram_tensor("ownT", [F, R], fp32, kind="ExternalInput").ap()
    b12 = nc.dram_tensor("b12", [1, 1], fp32, kind="ExternalInput").ap()
    biasv = nc.dram_tensor("biasv", [1, D], fp32, kind="ExternalInput").ap()
    out = nc.dram_tensor("out", [R, D], fp32, kind="ExternalOutput").ap()

    with tile.TileContext(nc) as tc:
        with ExitStack() as ctx:
            const = ctx.enter_context(tc.tile_pool(name="const", bufs=1))
            persist = ctx.enter_context(tc.tile_pool(name="persist", bufs=1))
            stp = ctx.enter_context(tc.tile_pool(name="stp", bufs=6))
            sqp = ctx.enter_context(tc.tile_pool(name="sqp", bufs=4))
            vp = ctx.enter_context(tc.tile_pool(name="vp", bufs=4))
            colp = ctx.enter_context(tc.tile_pool(name="colp", bufs=6))
            obp = ctx.enter_context(tc.tile_pool(name="obp", bufs=3))
            psp = ctx.enter_context(tc.tile_pool(name="psp", bufs=3, space="PSUM"))
            pvp = ctx.enter_context(tc.tile_pool(name="pvp", bufs=1, space="PSUM"))
            scrp = ctx.enter_context(tc.tile_pool(name="scrp", bufs=3, space="PSUM"))

            # ---- constants / parameters ----
            ra0 = const.tile([P, D + 2], fp32, name="ra0")
            ra1 = const.tile([P, D + 2], fp32, name="ra1")
            nc.sync.dma_start(ra0[:, :], ra[0:P, :])
            nc.sync.dma_start(ra1[:, :], ra[P : 2 * P, :])
            b12_sb = const.tile([1, 1], fp32, name="b12_sb")
            nc.sync.dma_start(b12_sb[:, :], b12[:, :])
            bias_sb = const.tile([1, D], fp32, name="bias_sb")
            nc.sync.dma_start(bias_sb[:, :], biasv[:, :])
            ones_row = const.tile([1, P], fp32, name="ones_row")
            nc.vector.memset(ones_row[:, :], 1.0)
            ident = const.tile([P, P], fp32, name="ident")
            make_identity(nc, ident[:, :])

            ot0 = const.tile([P, R], fp32, name="ot0")
            ot1 = const.tile([P, R], fp32, name="ot1")
            nc.sync.dma_start(ot0[:, :], ownT[0:P, :])
            nc.sync.dma_start(ot1[:, :], ownT[P : 2 * P, :])

            # ---- prologue: g1 row for own block, replicated m tile ----
            g1row = persist.tile([1, R], fp32, name="g1row")
            m_rep = persist.tile([P, R], fp32, name="m_rep")
            bias_rep = persist.tile([P, D], fp32, name="bias_rep")

            for h in range(2):
                pf = scrp.tile([P, 512], fp32, name=f"pf{h}", tag="scr")
                cs = slice(h * 512, (h + 1) * 512)
                nc.tensor.matmul(
                    pf[0:1, :], ra0[:, D : D + 1], ot0[:, cs], start=True, stop=False
                )
                nc.tensor.matmul(
                    pf[0:1, :], ra1[:, D : D + 1], ot1[:, cs], start=False, stop=True
                )
                # g1 = f1 + (b1 + b2)
                nc.scalar.activation(
                    g1row[0:1, cs], pf[0:1, :], AF.Identity, bias=b12_sb[0:1, 0:1]
                )
            # broadcast to 128 partitions, m = exp(0.8 g1)
            for h in range(2):
                pb = scrp.tile([P, 512], fp32, name=f"pb{h}", tag="scr")
                cs = slice(h * 512, (h + 1) * 512)
                nc.tensor.matmul(
                    pb[:, :], ones_row[:, :], g1row[0:1, cs], start=True, stop=True
                )
                nc.scalar.activation(m_rep[:, cs], pb[:, :], AF.Exp, scale=0.8)

            pbias = scrp.tile([P, 512], fp32, name="pbias", tag="scr")
            nc.tensor.matmul(
                pbias[:, 0:D], ones_row[:, :], bias_sb[0:1, :], start=True, stop=True
            )
            nc.vector.tensor_copy(bias_rep[:, :], pbias[:, 0:D])

            # ---- accumulators for vals^T ([a*seq_fts | a] contracted with w) ----
            pv0 = pvp.tile([D + 1, 512], fp32, name="pv0", tag="pv0")
            pv1 = pvp.tile([D + 1, 512], fp32, name="pv1", tag="pv1")

            # ---- main loop over j-chunks ----
            for jc in range(NJ):
                js = slice(jc * P, (jc + 1) * P)

                st0 = stp.tile([P, P], fp32, name=f"st0_{jc}", tag="st0")
                st1 = stp.tile([P, P], fp32, name=f"st1_{jc}", tag="st1")
                nc.sync.dma_start(st0[:, :], seqT[0:P, js])
                nc.sync.dma_start(st1[:, :], seqT[P : 2 * P, js])

                # seq_fts chunk + f1 + f2 columns: [128 j, 66]
                ps = psp.tile([P, D + 2], fp32, name=f"ps_{jc}", tag="ps")
                nc.tensor.matmul(ps[:, :], st0[:, :], ra0[:, :], start=True, stop=False)
                nc.tensor.matmul(ps[:, :], st1[:, :], ra1[:, :], start=False, stop=True)

                f2c = ps[:, D + 1 : D + 2]
                a_col = colp.tile([P, 1], fp32, name=f"a_{jc}", tag="a")
                c_col = colp.tile([P, 1], fp32, name=f"c_{jc}", tag="c")
                nc.scalar.activation(a_col[:, :], f2c, AF.Exp)
                nc.scalar.activation(c_col[:, :], f2c, AF.Exp, scale=-0.8)

                # sq = [a * seq_fts | a] in fp16: the mm_v matmul runs
                # 1 cyc/row in fp16 vs 4 cyc/row fp32; w rounding errors
                # appear in numerator AND denominator so they mostly cancel
                sq = sqp.tile([P, D + 1], fp16, name=f"sq_{jc}", tag="sq")
                nc.scalar.activation(sq[:, 0:D], ps[:, 0:D], AF.Copy, scale=a_col[:, :])
                nc.vector.tensor_copy(sq[:, D : D + 1], a_col[:, :])

                # w = max(m_i, c_j): one DVE tensor_scalar (2x perf mode)
                w = vp.tile([P, R], fp16, name=f"w_{jc}", tag="w")
                nc.vector.tensor_scalar_max(w[:, :], m_rep[:, :], c_col[:, :])

                first = jc == 0
                last = jc == NJ - 1
                nc.tensor.matmul(
                    pv0[:, :], sq[:, :], w[:, 0:512], start=first, stop=last
                )
                nc.tensor.matmul(
                    pv1[:, :], sq[:, :], w[:, 512:1024], start=first, stop=last
                )

            # ---- epilogue: transpose [65, 1024] -> 8x [128, 65], normalize ----
            vt = persist.tile([D + 1, R], fp32, name="vt")
            nc.scalar.activation(vt[:, 0:512], pv0[:, :], AF.Copy)
            nc.scalar.activation(vt[:, 512:1024], pv1[:, :], AF.Copy)

            for it in range(RI):
                cs = slice(it * P, (it + 1) * P)
                tp = psp.tile([P, D + 2], fp32, name=f"tp_{it}", tag="ps")
                nc.tensor.transpose(
                    tp[:, 0 : D + 1], vt[:, cs], ident[0 : D + 1, 0 : D + 1]
                )
                recip = colp.tile([P, 1], fp32, name=f"r_{it}", tag="r")
                nc.vector.reciprocal(recip[:, :], tp[:, D : D + 1])
                ob = obp.tile([P, D], fp32, name=f"ob_{it}", tag="ob")
                # out = vals_T * (1/denom) + bias
                nc.vector.scalar_tensor_tensor(
                    ob[:, :],
                    tp[:, 0:D],
                    recip[:, :],
                    bias_rep[:, :],
                    op0=OP.mult,
                    op1=OP.add,
                )
                nc.sync.dma_start(out[cs, :], ob[:, :])

    nc.compile()
    _prog_cache["nc"] = nc
    return nc


def _prep_inputs(seq, W0, w1, b1, w2, b2, bias):
    seq = np.asarray(seq, dtype=np.float32)
    W0 = np.asarray(W0, dtype=np.float32)
    w1 = np.asarray(w1, dtype=np.float32).reshape(D, 1)
    w2 = np.asarray(w2, dtype=np.float32).reshape(D, 1)
    b1 = np.asarray(b1, dtype=np.float32).reshape(-1)
    b2 = np.asarray(b2, dtype=np.float32).reshape(-1)
    bias = np.asarray(bias, dtype=np.float32).reshape(1, D)

    seqT = np.ascontiguousarray(seq.reshape(N, F).T)          # [F, N]
    ra = np.ascontiguousarray(
        np.concatenate([W0, W0 @ w1, W0 @ w2], axis=1)        # [F, D+2]
    )
    b12 = np.array([[b1[0] + b2[0]]], dtype=np.float32)

    in_maps = []
    for c in range(NCORES):
        ownT = np.ascontiguousarray(seqT[:, c * R : (c + 1) * R])
        in_maps.append(
            {"seqT": seqT, "ra": ra, "ownT": ownT, "b12": b12, "biasv": bias}
        )
    return in_maps


def run(inputs, trace=False):
    """Returns (output [1, N, D] float32, BassKernelResults)."""
    from concourse import bass_utils

    nc = _build_program()
    in_maps = _prep_inputs(**inputs)
    res = bass_utils.run_bass_kernel_spmd(
        nc, in_maps, core_ids=list(range(NCORES)), trace=trace
    )
    blocks = [res.results[c]["out"] for c in range(NCORES)]
    full = np.concatenate(blocks, axis=0).astype(np.float32)[None]  # [1, N, D]
    return full, res


def kernel(seq, W0, w1, b1, w2, b2, bias):
    out, _ = run(
        {
            "seq": seq,
            "W0": W0,
            "w1": w1,
            "b1": b1,
            "w2": w2,
            "b2": b2,
            "bias": bias,
        }
    )
    return out
